# revision 11
# baseline (speedup 1.0000x reference)
"""Trainium2 Bass kernel for nn_NodeModel (GNN message passing).

reference:
    agg = segment_sum(edge_attr, edge_index[0], num_segments=100000)   # [N, 64]
    h = concat([x, agg, u[v_indices]], axis=1)                         # [N, 256]
    out = relu(h @ W1 + b1) @ W2 + b2                                  # [N, 128]

Strategy (8 NeuronCores, SPMD, no collectives):
  - Shard nodes across cores (12500/core); shard edges by destination-node
    partition (host buckets+sorts edges by the core/window owning their row).
  - Nodes are processed in blocks of 128 (4 windows of 32). Edges are sorted
    by row, grouped per 32-node window, padded to Tb[slot] tiles of 128 edges.
    Since all 8 cores share one program, Tb is the max over cores; to cut the
    padding, each core's windows are rank-matched (sorted by edge count) so
    Tb[r] = ceil(max_c sorted_cnt[c][r]/128). The window->slot permutation is
    un-done on the host (x/u/out columns follow the same per-core permutation).
  - segment_sum on device: per 128-edge tile, one-hot P[e, m] = (idx[e] == m)
    built in batches of KB=8 tiles with ONE DVE op (broadcast APs), then
    TensorE matmul aggT += ea_tile.T @ P accumulated in PSUM.
  - Everything flows in bf16 (edge_attr, x, u-gather, W1) except the f32
    PSUM accumulators, the W2 matmul (fp32r) and the f32 output; rel err vs
    the f32 reference is ~5e-3, well under the 2e-2 gate.
  - MLP runs feature-major (transposed), N=512 node groups, interleaved with
    the edge phase (group g's MLP is emitted right after its 4 blocks) so
    there is no serial MLP tail. Output is un-transposed on host.
"""

import sys

sys.path.insert(0, "/opt/trn_rl_repo")

import numpy as np
import ml_dtypes

import concourse.bass as bass
import concourse.mybir as mybir
from concourse import bacc, tile
from concourse.bass_utils import run_bass_kernel_spmd

bf16 = ml_dtypes.bfloat16

D_X, D_E, D_U = 128, 64, 64
D_HID, D_OUT = 256, 128
NB = 128   # nodes per block
WIN = 4    # 32-node one-hot windows per block
W = 32     # window width
KB = 8     # one-hot tiles generated per DVE op

FULL_CFG = dict(n_cores=8, n_nodes=100000, npc=12500, blocks=98, group=4)

_cache = {}


def _build_nc(Tb, blocks, npad, group, n_cores=8, reps=1):
    """Build the SPMD Bass program. Tb = per-slot edge tile counts.

    reps > 1 wraps the whole computation in a hardware For_i loop — used
    only for timing (per-iteration time = delta(wall)/delta(reps), which
    cancels the host dispatch overhead)."""
    Tb = list(Tb)
    nwin = blocks * WIN
    assert len(Tb) == nwin
    offs = [0]
    for t in Tb:
        offs.append(offs[-1] + t)
    TT = offs[-1]
    max_blk_tiles = max(
        sum(Tb[b * WIN : (b + 1) * WIN]) for b in range(blocks)
    )
    nc = bacc.Bacc(
        "TRN2", target_bir_lowering=False, debug=False, num_devices=n_cores
    )
    f32, rf32, b16 = mybir.dt.float32, mybir.dt.float32r, mybir.dt.bfloat16

    # partition-major: partition = edge slot within tile, free = (tile, feat)
    ea_in = nc.declare_dram_parameter("ea", [128, TT * 64], b16, isOutput=False)
    idx_in = nc.declare_dram_parameter("idx", [128, TT], b16, isOutput=False)
    iota_in = nc.declare_dram_parameter("iota", [128, KB, W], b16, isOutput=False)
    xT_in = nc.declare_dram_parameter("xT", [128, npad], b16, isOutput=False)
    ugT_in = nc.declare_dram_parameter("ugT", [64, npad], b16, isOutput=False)
    # weight layouts are partition-major: [K-part, mh, M]
    w1x_in = nc.declare_dram_parameter("w1x", [128, 2, 128], b16, isOutput=False)
    w1a_in = nc.declare_dram_parameter("w1a", [64, 2, 128], b16, isOutput=False)
    w1u_in = nc.declare_dram_parameter("w1u", [64, 2, 128], b16, isOutput=False)
    w2_in = nc.declare_dram_parameter("w2", [128, 2, 128], rf32, isOutput=False)
    b1_in = nc.declare_dram_parameter("b1", [128, 2], f32, isOutput=False)
    b2_in = nc.declare_dram_parameter("b2", [128, 1], f32, isOutput=False)
    outT = nc.declare_dram_parameter("outT", [128, npad], f32, isOutput=True)

    n_groups = (blocks + group - 1) // group
    GNB = group * NB

    with tile.TileContext(nc) as tc:
        with (
            tc.tile_pool(name="const", bufs=1) as cpool,
            tc.tile_pool(name="ea", bufs=6) as eapool,
            tc.tile_pool(name="p", bufs=6) as ppool,
            tc.tile_pool(name="hag", bufs=3) as hagpool,
            tc.tile_pool(name="ug", bufs=2) as ugpool,
            tc.tile_pool(name="h1", bufs=4) as h1pool,
            tc.tile_pool(name="outs", bufs=2) as opool,
            tc.tile_pool(name="ps_agg", bufs=4, space="PSUM") as agg_ps_pool,
            tc.tile_pool(name="ps_o1", bufs=2, space="PSUM") as o1_ps_pool,
            tc.tile_pool(name="ps_o2", bufs=2, space="PSUM") as o2_ps_pool,
        ):
          def _emit_body():
              # ---- constants / resident tensors (cheap Pool-ring DMAs) ----
              iota_t = cpool.tile([128, KB, W], b16, tag="iota")
              nc.gpsimd.dma_start(iota_t[:], iota_in[:])
              idx_t = cpool.tile([128, TT], b16, tag="idx")
              nc.gpsimd.dma_start(idx_t[:], idx_in[:])
              w1x_t = cpool.tile([128, 2, 128], b16, tag="w1x")
              nc.gpsimd.dma_start(w1x_t[:], w1x_in[:])
              w1a_t = cpool.tile([64, 2, 128], b16, tag="w1a")
              nc.gpsimd.dma_start(w1a_t[:], w1a_in[:])
              w1u_t = cpool.tile([64, 2, 128], b16, tag="w1u")
              nc.gpsimd.dma_start(w1u_t[:], w1u_in[:])
              w2_t = cpool.tile([128, 2, 128], rf32, tag="w2")
              nc.gpsimd.dma_start(w2_t[:], w2_in[:])
              b1_t = cpool.tile([128, 2], f32, tag="b1")
              nc.gpsimd.dma_start(b1_t[:], b1_in[:])
              b2_t = cpool.tile([128, 1], f32, tag="b2")
              nc.gpsimd.dma_start(b2_t[:], b2_in[:])

              xT_t = cpool.tile([128, npad], b16, tag="xT")
              # load x in chunks so early groups can start sooner
              xchunk = 8 * NB
              for s in range(0, npad, xchunk):
                  e = min(s + xchunk, npad)
                  nc.gpsimd.dma_start(xT_t[:, s:e], xT_in[:, s:e])

              hag_tiles = {}
              ug_tiles = {}
              p_cur = None

              def emit_mlp(g):
                  s = g * GNB
                  gw = min(GNB, npad - s)
                  hag = hag_tiles.pop(g)
                  ug_t = ug_tiles.pop(g)
                  h1s = []
                  for mh in range(2):
                      o1 = o1_ps_pool.tile([128, GNB], f32, tag="o1")
                      nc.tensor.matmul(
                          o1[:, :gw], w1x_t[:, mh, :], xT_t[:, s : s + gw],
                          start=True, stop=False,
                      )
                      nc.tensor.matmul(
                          o1[:, :gw], w1a_t[:, mh, :], hag[:, :gw],
                          start=False, stop=False,
                      )
                      nc.tensor.matmul(
                          o1[:, :gw], w1u_t[:, mh, :], ug_t[:, :gw],
                          start=False, stop=True,
                      )
                      h1 = h1pool.tile([128, GNB], rf32, tag="h1")
                      nc.scalar.activation(
                          out=h1[:, :gw], in_=o1[:, :gw],
                          func=mybir.ActivationFunctionType.Relu,
                          bias=b1_t[:, mh : mh + 1],
                      )
                      h1s.append(h1)
                  o2 = o2_ps_pool.tile([128, GNB], f32, tag="o2")
                  for kh in range(2):
                      nc.tensor.matmul(
                          o2[:, :gw], w2_t[:, kh, :], h1s[kh][:, :gw],
                          start=(kh == 0), stop=(kh == 1),
                      )
                  out_t = opool.tile([128, GNB], f32, tag="outs")
                  nc.scalar.activation(
                      out=out_t[:, :gw], in_=o2[:, :gw],
                      func=mybir.ActivationFunctionType.Identity,
                      bias=b2_t[:],
                  )
                  nc.scalar.dma_start(outT[:, s : s + gw], out_t[:, :gw])

              # ---- edge scatter-add per block, MLP interleaved per group ----
              ea_tiles = {}
              max_grp_tiles = max(
                  offs[min((g + 1) * group, blocks) * WIN] - offs[g * group * WIN]
                  for g in range(n_groups)
              )
              for b in range(blocks):
                  g, bi = divmod(b, group)
                  nblk_g = min(group, blocks - g * group)
                  if bi == 0:
                      hag_tiles[g] = hagpool.tile(
                          [64, GNB], b16, tag="hag", name=f"hag{g}"
                      )
                      s = g * GNB
                      gw = min(GNB, npad - s)
                      ug_t = ugpool.tile([64, GNB], b16, tag="ug", name=f"ug{g}")
                      nc.scalar.dma_start(ug_t[:, :gw], ugT_in[:, s : s + gw])
                      ug_tiles[g] = ug_t
                  o_g = offs[b * WIN]
                  Tblk = offs[(b + 1) * WIN] - o_g
                  ea_t = eapool.tile(
                      [128, max_blk_tiles * 64], b16, tag="ea", name=f"ea{b}"
                  )
                  ea_ring = nc.sync if b % 2 == 0 else nc.gpsimd
                  ea_ring.dma_start(
                      ea_t[:, : Tblk * 64],
                      ea_in[:, o_g * 64 : (o_g + Tblk) * 64],
                  )
                  agg_ps = agg_ps_pool.tile([64, NB], f32, tag="agg")
                  for w in range(WIN):
                      r = b * WIN + w
                      for t in range(Tb[r]):
                          o = offs[r] + t
                          if o % KB == 0:
                              kk = min(KB, TT - o)
                              p_cur = ppool.tile([128, KB, W], b16, tag="p")
                              idx_b = (
                                  idx_t[:, o : o + kk]
                                  .unsqueeze(2)
                                  .broadcast_to([128, kk, W])
                              )
                              nc.vector.scalar_tensor_tensor(
                                  out=p_cur[:, :kk, :],
                                  in0=iota_t[:, :kk, :],
                                  scalar=1.0,
                                  in1=idx_b,
                                  op0=mybir.AluOpType.mult,
                                  op1=mybir.AluOpType.is_equal,
                              )
                          nc.tensor.matmul(
                              agg_ps[:, W * w : W * (w + 1)],
                              ea_t[:, (o - o_g) * 64 : (o - o_g + 1) * 64],
                              p_cur[:, o % KB, :],
                              start=(t == 0),
                              stop=(t == Tb[r] - 1),
                          )
                  # stage [64, NB] agg into the group's MLP input tile (bf16)
                  nc.scalar.activation(
                      out=hag_tiles[g][:, bi * NB : (bi + 1) * NB],
                      in_=agg_ps[:],
                      func=mybir.ActivationFunctionType.Copy,
                  )
                  if bi == nblk_g - 1:
                      emit_mlp(g)

          if reps == 1:
              _emit_body()
          else:
              with tc.For_i(0, reps, 1):
                  _emit_body()

    nc.compile()
    return nc


def _pack_inputs(x, edge_index, edge_attr, u, v_indices, W1, b1, W2, b2, cfg):
    """Host-side sharding: bucket + sort edges by destination node window.

    Returns (in_maps, Tb, ids) where ids[c][j] is the global node id behind
    column j of core c's transposed layout (-1 for padding columns)."""
    n_cores, npc, blocks = cfg["n_cores"], cfg["npc"], cfg["blocks"]
    n_nodes = cfg["n_nodes"]
    npad = blocks * NB
    nwin = blocks * WIN
    row = np.asarray(edge_index[0], dtype=np.int64)
    ea = np.ascontiguousarray(np.asarray(edge_attr, dtype=np.float32))
    x = np.asarray(x, dtype=np.float32)
    u = np.asarray(u, dtype=np.float32)
    v_indices = np.asarray(v_indices, dtype=np.int64)
    W1 = np.asarray(W1, dtype=np.float32)
    W2 = np.asarray(W2, dtype=np.float32)
    b1 = np.asarray(b1, dtype=np.float32)
    b2 = np.asarray(b2, dtype=np.float32)
    d_e = ea.shape[1]

    order = np.argsort(row, kind="stable")
    row_s = row[order]
    ea_s = ea[order].astype(bf16)

    # window boundaries: core c window i covers nodes [npc*c + 32*i, +32),
    # clipped to the core's node range.
    bases = (npc * np.arange(n_cores)[:, None] + W * np.arange(nwin)[None, :]).ravel()
    core_hi = (npc * (1 + np.arange(n_cores))[:, None]).repeat(nwin, 1).ravel()
    starts = np.searchsorted(row_s, np.minimum(bases, core_hi), side="left")
    ends = np.searchsorted(row_s, np.minimum(bases + W, core_hi), side="left")
    cnts = (ends - starts).reshape(n_cores, nwin)

    # rank-match: each core sorts its windows by count desc; slot r on every
    # core holds that core's r-th largest window, so the shared Tb is tight.
    ordw = np.argsort(-cnts, axis=1, kind="stable")          # [n_cores, nwin]
    cnt_sorted = np.take_along_axis(cnts, ordw, axis=1)
    mx = cnt_sorted.max(axis=0)                               # [nwin]
    Tb = np.maximum(1, -(-mx // 128)).astype(int)
    offs = np.concatenate([[0], np.cumsum(Tb)])
    TT = int(offs[-1])

    uT = u.T  # [d_u, n_graphs]
    starts2 = starts.reshape(n_cores, nwin)
    ends2 = ends.reshape(n_cores, nwin)

    in_maps = []
    ids_list = []
    iota = np.broadcast_to(
        np.arange(W, dtype=np.float32), (128, KB, W)
    ).astype(bf16)
    # weights, partition-major [K, mh, M]
    w1x = np.ascontiguousarray(W1[:D_X].reshape(D_X, 2, 128)).astype(bf16)
    w1a = np.ascontiguousarray(W1[D_X : D_X + d_e].reshape(d_e, 2, 128)).astype(bf16)
    w1u = np.ascontiguousarray(W1[D_X + d_e :].reshape(D_U, 2, 128)).astype(bf16)
    w2 = np.ascontiguousarray(W2.reshape(2, 128, D_OUT).transpose(1, 0, 2))
    b1p = np.ascontiguousarray(b1.reshape(2, 128).T)
    b2p = np.ascontiguousarray(b2.reshape(128, 1))

    for c in range(n_cores):
        cnt = cnts[c]
        cs, ce = starts2[c, 0], ends2[c, -1]
        slotof = np.empty(nwin, dtype=np.int64)
        slotof[ordw[c]] = np.arange(nwin)
        w_e = np.repeat(np.arange(nwin), cnt)            # window id per edge
        rank = np.arange(ce - cs) - np.repeat(starts2[c] - cs, cnt)
        slot_idx = offs[slotof[w_e]] * 128 + rank
        coreslots = np.zeros((TT * 128, d_e), dtype=bf16)
        coreslots[slot_idx] = ea_s[cs:ce]
        ea_pack = (
            coreslots.reshape(TT, 128, d_e).transpose(1, 0, 2).reshape(128, TT * d_e)
        )
        ivals = np.zeros(TT * 128, dtype=np.float32)
        ivals[slot_idx] = (row_s[cs:ce] - (npc * c + W * w_e)).astype(np.float32)
        idx_pack = np.ascontiguousarray(ivals.reshape(TT, 128).T).astype(bf16)

        base_nodes = npc * c + W * ordw[c]                # [nwin]
        ids = (base_nodes[:, None] + np.arange(W)).ravel()  # [npad]
        valid = ids < min(npc * (c + 1), n_nodes)
        ids_eff = np.where(valid, ids, 0)
        xT = np.where(valid[None, :], x[ids_eff].T, 0.0).astype(bf16)
        ugT = np.where(valid[None, :], uT[:, v_indices[ids_eff]], 0.0).astype(bf16)
        ids_list.append(np.where(valid, ids, -1))
        in_maps.append({
            "ea": ea_pack,
            "idx": idx_pack,
            "iota": iota,
            "xT": np.ascontiguousarray(xT),
            "ugT": np.ascontiguousarray(ugT),
            "w1x": w1x,
            "w1a": w1a,
            "w1u": w1u,
            "w2": w2,
            "b1": b1p,
            "b2": b2p,
        })
    return in_maps, tuple(int(t) for t in Tb), ids_list


def unpack_out(outT_list, ids_list, n_nodes=100000):
    out = np.empty((n_nodes, D_OUT), dtype=np.float32)
    for c, ids in enumerate(ids_list):
        valid = ids >= 0
        out[ids[valid]] = outT_list[c].T[valid]
    return out


def _run(inputs, cfg, trace=False, reps=1):
    in_maps, T, ids_list = _pack_inputs(
        inputs["x"], inputs["edge_index"], inputs["edge_attr"], inputs["u"],
        inputs["v_indices"], inputs["W1"], inputs["b1"], inputs["W2"],
        inputs["b2"], cfg,
    )
    key = (T, cfg["blocks"], cfg["group"], reps)
    if key not in _cache:
        _cache[key] = _build_nc(
            T, cfg["blocks"], cfg["blocks"] * NB, cfg["group"], reps=reps
        )
    nc = _cache[key]
    res = run_bass_kernel_spmd(nc, in_maps, list(range(cfg["n_cores"])), trace=trace)
    out = unpack_out(
        [res.results[c]["outT"] for c in range(cfg["n_cores"])],
        ids_list, cfg["n_nodes"],
    )
    _run.last_results = res
    return out


def kernel(x, edge_index, edge_attr, u, v_indices, W1, b1, W2, b2):
    inputs = dict(x=x, edge_index=edge_index, edge_attr=edge_attr, u=u,
                  v_indices=v_indices, W1=W1, b1=b1, W2=W2, b2=b2)
    return _run(inputs, FULL_CFG)


# revision 12
# speedup vs baseline: 1.0723x; 1.0723x over previous
"""Trainium2 Bass kernel for nn_NodeModel (GNN message passing).

reference:
    agg = segment_sum(edge_attr, edge_index[0], num_segments=100000)   # [N, 64]
    h = concat([x, agg, u[v_indices]], axis=1)                         # [N, 256]
    out = relu(h @ W1 + b1) @ W2 + b2                                  # [N, 128]

Strategy (8 NeuronCores, SPMD, no collectives):
  - Shard nodes across cores (12500/core); shard edges by destination-node
    partition (host buckets+sorts edges by the core/window owning their row).
  - Nodes are processed in blocks of 128 (4 windows of 32). Edges are sorted
    by row, grouped per 32-node window, padded to Tb[slot] tiles of 128 edges.
    Since all 8 cores share one program, Tb is the max over cores; to cut the
    padding, each core's windows are rank-matched (sorted by edge count) so
    Tb[r] = ceil(max_c sorted_cnt[c][r]/128). The window->slot permutation is
    un-done on the host (x/u/out columns follow the same per-core permutation).
  - segment_sum on device: per 128-edge tile, one-hot P[e, m] = (idx[e] == m)
    built in batches of KB=8 tiles with ONE DVE op (broadcast APs), then
    TensorE matmul aggT += ea_tile.T @ P accumulated in PSUM.
  - Everything flows in bf16 (edge_attr, x, u-gather, W1) except the f32
    PSUM accumulators, the W2 matmul (fp32r) and the f32 output; rel err vs
    the f32 reference is ~5e-3, well under the 2e-2 gate.
  - MLP runs feature-major (transposed), N=512 node groups, interleaved with
    the edge phase (group g's MLP is emitted right after its 4 blocks) so
    there is no serial MLP tail. Output is un-transposed on host.
"""

import sys

sys.path.insert(0, "/opt/trn_rl_repo")

import numpy as np
import ml_dtypes

import concourse.bass as bass
import concourse.mybir as mybir
from concourse import bacc, tile
from concourse.bass_utils import run_bass_kernel_spmd

bf16 = ml_dtypes.bfloat16

D_X, D_E, D_U = 128, 64, 64
D_HID, D_OUT = 256, 128
NB = 128   # nodes per block
WIN = 4    # 32-node one-hot windows per block
W = 32     # window width
KB = 8     # one-hot tiles generated per DVE op

FULL_CFG = dict(n_cores=8, n_nodes=100000, npc=12500, blocks=98, group=4)

_cache = {}


def _build_nc(Tb, blocks, npad, group, n_cores=8, reps=1):
    """Build the SPMD Bass program. Tb = per-slot edge tile counts.

    reps > 1 wraps the whole computation in a hardware For_i loop — used
    only for timing (per-iteration time = delta(wall)/delta(reps), which
    cancels the host dispatch overhead)."""
    Tb = list(Tb)
    nwin = blocks * WIN
    assert len(Tb) == nwin
    offs = [0]
    for t in Tb:
        offs.append(offs[-1] + t)
    TT = offs[-1]
    max_blk_tiles = max(
        sum(Tb[b * WIN : (b + 1) * WIN]) for b in range(blocks)
    )
    nc = bacc.Bacc(
        "TRN2", target_bir_lowering=False, debug=False, num_devices=n_cores
    )
    f32, rf32, b16 = mybir.dt.float32, mybir.dt.float32r, mybir.dt.bfloat16

    # partition-major: partition = edge slot within tile, free = (tile, feat)
    ea_in = nc.declare_dram_parameter("ea", [128, TT * 64], b16, isOutput=False)
    idx_in = nc.declare_dram_parameter("idx", [128, TT], b16, isOutput=False)
    iota_in = nc.declare_dram_parameter("iota", [128, KB, W], b16, isOutput=False)
    xT_in = nc.declare_dram_parameter("xT", [128, npad], b16, isOutput=False)
    ugT_in = nc.declare_dram_parameter("ugT", [64, npad], b16, isOutput=False)
    # weight layouts are partition-major: [K-part, mh, M]
    w1x_in = nc.declare_dram_parameter("w1x", [128, 2, 128], b16, isOutput=False)
    w1a_in = nc.declare_dram_parameter("w1a", [64, 2, 128], b16, isOutput=False)
    w1u_in = nc.declare_dram_parameter("w1u", [64, 2, 128], b16, isOutput=False)
    w2_in = nc.declare_dram_parameter("w2", [128, 2, 128], rf32, isOutput=False)
    b1_in = nc.declare_dram_parameter("b1", [128, 2], f32, isOutput=False)
    b2_in = nc.declare_dram_parameter("b2", [128, 1], f32, isOutput=False)
    outT = nc.declare_dram_parameter("outT", [128, npad], f32, isOutput=True)

    n_groups = (blocks + group - 1) // group
    GNB = group * NB

    with tile.TileContext(nc) as tc:
        with (
            tc.tile_pool(name="const", bufs=1) as cpool,
            tc.tile_pool(name="ea", bufs=6) as eapool,
            tc.tile_pool(name="p", bufs=6) as ppool,
            tc.tile_pool(name="hag", bufs=3) as hagpool,
            tc.tile_pool(name="ug", bufs=2) as ugpool,
            tc.tile_pool(name="h1", bufs=4) as h1pool,
            tc.tile_pool(name="outs", bufs=2) as opool,
            tc.tile_pool(name="ps_agg", bufs=4, space="PSUM") as agg_ps_pool,
            tc.tile_pool(name="ps_o1", bufs=2, space="PSUM") as o1_ps_pool,
            tc.tile_pool(name="ps_o2", bufs=2, space="PSUM") as o2_ps_pool,
        ):
          def _emit_body():
              # ---- constants / resident tensors (cheap Pool-ring DMAs) ----
              iota_t = cpool.tile([128, KB, W], b16, tag="iota")
              nc.gpsimd.dma_start(iota_t[:], iota_in[:])
              idx_t = cpool.tile([128, TT], b16, tag="idx")
              nc.gpsimd.dma_start(idx_t[:], idx_in[:])
              w1x_t = cpool.tile([128, 2, 128], b16, tag="w1x")
              nc.gpsimd.dma_start(w1x_t[:], w1x_in[:])
              w1a_t = cpool.tile([64, 2, 128], b16, tag="w1a")
              nc.gpsimd.dma_start(w1a_t[:], w1a_in[:])
              w1u_t = cpool.tile([64, 2, 128], b16, tag="w1u")
              nc.gpsimd.dma_start(w1u_t[:], w1u_in[:])
              w2_t = cpool.tile([128, 2, 128], rf32, tag="w2")
              nc.gpsimd.dma_start(w2_t[:], w2_in[:])
              b1_t = cpool.tile([128, 2], f32, tag="b1")
              nc.gpsimd.dma_start(b1_t[:], b1_in[:])
              b2_t = cpool.tile([128, 1], f32, tag="b2")
              nc.gpsimd.dma_start(b2_t[:], b2_in[:])

              xT_t = cpool.tile([128, npad], b16, tag="xT")
              # load x in chunks so early groups can start sooner
              xchunk = 25 * NB
              for s in range(0, npad, xchunk):
                  e = min(s + xchunk, npad)
                  nc.gpsimd.dma_start(xT_t[:, s:e], xT_in[:, s:e])

              hag_tiles = {}
              ug_tiles = {}
              p_cur = None

              def emit_mlp(g):
                  s = g * GNB
                  gw = min(GNB, npad - s)
                  hag = hag_tiles.pop(g)
                  ug_t, uoff = ug_tiles.pop(g)
                  h1s = []
                  for mh in range(2):
                      o1 = o1_ps_pool.tile([128, GNB], f32, tag="o1")
                      nc.tensor.matmul(
                          o1[:, :gw], w1x_t[:, mh, :], xT_t[:, s : s + gw],
                          start=True, stop=False,
                      )
                      nc.tensor.matmul(
                          o1[:, :gw], w1a_t[:, mh, :], hag[:, :gw],
                          start=False, stop=False,
                      )
                      nc.tensor.matmul(
                          o1[:, :gw], w1u_t[:, mh, :], ug_t[:, uoff : uoff + gw],
                          start=False, stop=True,
                      )
                      h1 = h1pool.tile([128, GNB], rf32, tag="h1")
                      nc.scalar.activation(
                          out=h1[:, :gw], in_=o1[:, :gw],
                          func=mybir.ActivationFunctionType.Relu,
                          bias=b1_t[:, mh : mh + 1],
                      )
                      h1s.append(h1)
                  o2 = o2_ps_pool.tile([128, GNB], f32, tag="o2")
                  for kh in range(2):
                      nc.tensor.matmul(
                          o2[:, :gw], w2_t[:, kh, :], h1s[kh][:, :gw],
                          start=(kh == 0), stop=(kh == 1),
                      )
                  out_t = opool.tile([128, GNB], f32, tag="outs")
                  nc.scalar.activation(
                      out=out_t[:, :gw], in_=o2[:, :gw],
                      func=mybir.ActivationFunctionType.Identity,
                      bias=b2_t[:],
                  )
                  nc.scalar.dma_start(outT[:, s : s + gw], out_t[:, :gw])

              # ---- edge scatter-add per block, MLP interleaved per group ----
              ea_tiles = {}
              max_grp_tiles = max(
                  offs[min((g + 1) * group, blocks) * WIN] - offs[g * group * WIN]
                  for g in range(n_groups)
              )
              for b in range(blocks):
                  g, bi = divmod(b, group)
                  nblk_g = min(group, blocks - g * group)
                  if bi == 0:
                      hag_tiles[g] = hagpool.tile(
                          [64, GNB], b16, tag="hag", name=f"hag{g}"
                      )
                      if g % 2 == 0:
                          s2 = g * GNB
                          gw2 = min(2 * GNB, npad - s2)
                          ug_t = ugpool.tile(
                              [64, 2 * GNB], b16, tag="ug", name=f"ug{g}"
                          )
                          nc.gpsimd.dma_start(
                              ug_t[:, :gw2], ugT_in[:, s2 : s2 + gw2]
                          )
                          ug_tiles[g] = (ug_t, 0)
                          if g + 1 < n_groups:
                              ug_tiles[g + 1] = (ug_t, GNB)
                  o_g = offs[b * WIN]
                  Tblk = offs[(b + 1) * WIN] - o_g
                  ea_t = eapool.tile(
                      [128, max_blk_tiles * 64], b16, tag="ea", name=f"ea{b}"
                  )
                  ea_ring = nc.sync if b % 4 != 3 else nc.gpsimd
                  ea_ring.dma_start(
                      ea_t[:, : Tblk * 64],
                      ea_in[:, o_g * 64 : (o_g + Tblk) * 64],
                  )
                  agg_ps = agg_ps_pool.tile([64, NB], f32, tag="agg")
                  for w in range(WIN):
                      r = b * WIN + w
                      for t in range(Tb[r]):
                          o = offs[r] + t
                          if o % KB == 0:
                              kk = min(KB, TT - o)
                              p_cur = ppool.tile([128, KB, W], b16, tag="p")
                              idx_b = (
                                  idx_t[:, o : o + kk]
                                  .unsqueeze(2)
                                  .broadcast_to([128, kk, W])
                              )
                              nc.vector.scalar_tensor_tensor(
                                  out=p_cur[:, :kk, :],
                                  in0=iota_t[:, :kk, :],
                                  scalar=1.0,
                                  in1=idx_b,
                                  op0=mybir.AluOpType.mult,
                                  op1=mybir.AluOpType.is_equal,
                              )
                          nc.tensor.matmul(
                              agg_ps[:, W * w : W * (w + 1)],
                              ea_t[:, (o - o_g) * 64 : (o - o_g + 1) * 64],
                              p_cur[:, o % KB, :],
                              start=(t == 0),
                              stop=(t == Tb[r] - 1),
                          )
                  # stage [64, NB] agg into the group's MLP input tile (bf16)
                  nc.scalar.activation(
                      out=hag_tiles[g][:, bi * NB : (bi + 1) * NB],
                      in_=agg_ps[:],
                      func=mybir.ActivationFunctionType.Copy,
                  )
                  if bi == nblk_g - 1:
                      emit_mlp(g)

          if reps == 1:
              _emit_body()
          else:
              with tc.For_i(0, reps, 1):
                  _emit_body()

    nc.compile()
    return nc


def _pack_inputs(x, edge_index, edge_attr, u, v_indices, W1, b1, W2, b2, cfg):
    """Host-side sharding: bucket + sort edges by destination node window.

    Returns (in_maps, Tb, ids) where ids[c][j] is the global node id behind
    column j of core c's transposed layout (-1 for padding columns)."""
    n_cores, npc, blocks = cfg["n_cores"], cfg["npc"], cfg["blocks"]
    n_nodes = cfg["n_nodes"]
    npad = blocks * NB
    nwin = blocks * WIN
    row = np.asarray(edge_index[0], dtype=np.int64)
    ea = np.ascontiguousarray(np.asarray(edge_attr, dtype=np.float32))
    x = np.asarray(x, dtype=np.float32)
    u = np.asarray(u, dtype=np.float32)
    v_indices = np.asarray(v_indices, dtype=np.int64)
    W1 = np.asarray(W1, dtype=np.float32)
    W2 = np.asarray(W2, dtype=np.float32)
    b1 = np.asarray(b1, dtype=np.float32)
    b2 = np.asarray(b2, dtype=np.float32)
    d_e = ea.shape[1]

    order = np.argsort(row, kind="stable")
    row_s = row[order]
    ea_s = ea[order].astype(bf16)

    # window boundaries: core c window i covers nodes [npc*c + 32*i, +32),
    # clipped to the core's node range.
    bases = (npc * np.arange(n_cores)[:, None] + W * np.arange(nwin)[None, :]).ravel()
    core_hi = (npc * (1 + np.arange(n_cores))[:, None]).repeat(nwin, 1).ravel()
    starts = np.searchsorted(row_s, np.minimum(bases, core_hi), side="left")
    ends = np.searchsorted(row_s, np.minimum(bases + W, core_hi), side="left")
    cnts = (ends - starts).reshape(n_cores, nwin)

    # rank-match: each core sorts its windows by count desc; slot r on every
    # core holds that core's r-th largest window, so the shared Tb is tight.
    ordw = np.argsort(-cnts, axis=1, kind="stable")          # [n_cores, nwin]
    cnt_sorted = np.take_along_axis(cnts, ordw, axis=1)
    mx = cnt_sorted.max(axis=0)                               # [nwin]
    Tb = np.maximum(1, -(-mx // 128)).astype(int)
    offs = np.concatenate([[0], np.cumsum(Tb)])
    TT = int(offs[-1])

    uT = u.T  # [d_u, n_graphs]
    starts2 = starts.reshape(n_cores, nwin)
    ends2 = ends.reshape(n_cores, nwin)

    in_maps = []
    ids_list = []
    iota = np.broadcast_to(
        np.arange(W, dtype=np.float32), (128, KB, W)
    ).astype(bf16)
    # weights, partition-major [K, mh, M]
    w1x = np.ascontiguousarray(W1[:D_X].reshape(D_X, 2, 128)).astype(bf16)
    w1a = np.ascontiguousarray(W1[D_X : D_X + d_e].reshape(d_e, 2, 128)).astype(bf16)
    w1u = np.ascontiguousarray(W1[D_X + d_e :].reshape(D_U, 2, 128)).astype(bf16)
    w2 = np.ascontiguousarray(W2.reshape(2, 128, D_OUT).transpose(1, 0, 2))
    b1p = np.ascontiguousarray(b1.reshape(2, 128).T)
    b2p = np.ascontiguousarray(b2.reshape(128, 1))

    for c in range(n_cores):
        cnt = cnts[c]
        cs, ce = starts2[c, 0], ends2[c, -1]
        slotof = np.empty(nwin, dtype=np.int64)
        slotof[ordw[c]] = np.arange(nwin)
        w_e = np.repeat(np.arange(nwin), cnt)            # window id per edge
        rank = np.arange(ce - cs) - np.repeat(starts2[c] - cs, cnt)
        slot_idx = offs[slotof[w_e]] * 128 + rank
        coreslots = np.zeros((TT * 128, d_e), dtype=bf16)
        coreslots[slot_idx] = ea_s[cs:ce]
        ea_pack = (
            coreslots.reshape(TT, 128, d_e).transpose(1, 0, 2).reshape(128, TT * d_e)
        )
        ivals = np.zeros(TT * 128, dtype=np.float32)
        ivals[slot_idx] = (row_s[cs:ce] - (npc * c + W * w_e)).astype(np.float32)
        idx_pack = np.ascontiguousarray(ivals.reshape(TT, 128).T).astype(bf16)

        base_nodes = npc * c + W * ordw[c]                # [nwin]
        ids = (base_nodes[:, None] + np.arange(W)).ravel()  # [npad]
        valid = ids < min(npc * (c + 1), n_nodes)
        ids_eff = np.where(valid, ids, 0)
        xT = np.where(valid[None, :], x[ids_eff].T, 0.0).astype(bf16)
        ugT = np.where(valid[None, :], uT[:, v_indices[ids_eff]], 0.0).astype(bf16)
        ids_list.append(np.where(valid, ids, -1))
        in_maps.append({
            "ea": ea_pack,
            "idx": idx_pack,
            "iota": iota,
            "xT": np.ascontiguousarray(xT),
            "ugT": np.ascontiguousarray(ugT),
            "w1x": w1x,
            "w1a": w1a,
            "w1u": w1u,
            "w2": w2,
            "b1": b1p,
            "b2": b2p,
        })
    return in_maps, tuple(int(t) for t in Tb), ids_list


def unpack_out(outT_list, ids_list, n_nodes=100000):
    out = np.empty((n_nodes, D_OUT), dtype=np.float32)
    for c, ids in enumerate(ids_list):
        valid = ids >= 0
        out[ids[valid]] = outT_list[c].T[valid]
    return out


def _run(inputs, cfg, trace=False, reps=1):
    in_maps, T, ids_list = _pack_inputs(
        inputs["x"], inputs["edge_index"], inputs["edge_attr"], inputs["u"],
        inputs["v_indices"], inputs["W1"], inputs["b1"], inputs["W2"],
        inputs["b2"], cfg,
    )
    key = (T, cfg["blocks"], cfg["group"], reps)
    if key not in _cache:
        _cache[key] = _build_nc(
            T, cfg["blocks"], cfg["blocks"] * NB, cfg["group"], reps=reps
        )
    nc = _cache[key]
    res = run_bass_kernel_spmd(nc, in_maps, list(range(cfg["n_cores"])), trace=trace)
    out = unpack_out(
        [res.results[c]["outT"] for c in range(cfg["n_cores"])],
        ids_list, cfg["n_nodes"],
    )
    _run.last_results = res
    return out


def kernel(x, edge_index, edge_attr, u, v_indices, W1, b1, W2, b2):
    inputs = dict(x=x, edge_index=edge_index, edge_attr=edge_attr, u=u,
                  v_indices=v_indices, W1=W1, b1=b1, W2=W2, b2=b2)
    return _run(inputs, FULL_CFG)


# revision 14
# speedup vs baseline: 1.1984x; 1.1177x over previous
"""Trainium2 Bass kernel for nn_NodeModel (GNN message passing).

reference:
    agg = segment_sum(edge_attr, edge_index[0], num_segments=100000)   # [N, 64]
    h = concat([x, agg, u[v_indices]], axis=1)                         # [N, 256]
    out = relu(h @ W1 + b1) @ W2 + b2                                  # [N, 128]

Strategy (8 NeuronCores, SPMD, no collectives):
  - Shard nodes across cores (12500/core); shard edges by destination-node
    partition (host buckets+sorts edges by the core/window owning their row).
  - Nodes are processed in blocks of 128 (4 windows of 32). Edges are sorted
    by row, grouped per 32-node window, padded to Tb[slot] tiles of 128 edges.
    Since all 8 cores share one program, Tb is the max over cores; to cut the
    padding, each core's windows are rank-matched (sorted by edge count) so
    Tb[r] = ceil(max_c sorted_cnt[c][r]/128). The window->slot permutation is
    un-done on the host (x/u/out columns follow the same per-core permutation).
  - segment_sum on device: per 128-edge tile, one-hot P[e, m] = (idx[e] == m)
    built in batches of KB=8 tiles with ONE DVE op (broadcast APs), then
    TensorE matmul aggT += ea_tile.T @ P accumulated in PSUM.
  - Everything flows in bf16 (edge_attr, x, u-gather, W1) except the f32
    PSUM accumulators, the W2 matmul (fp32r) and the f32 output; rel err vs
    the f32 reference is ~5e-3, well under the 2e-2 gate.
  - MLP runs feature-major (transposed), N=512 node groups, interleaved with
    the edge phase (group g's MLP is emitted right after its 4 blocks) so
    there is no serial MLP tail. Output is un-transposed on host.
"""

import sys

sys.path.insert(0, "/opt/trn_rl_repo")

import numpy as np
import ml_dtypes

import concourse.bass as bass
import concourse.mybir as mybir
from concourse import bacc, tile
from concourse.bass_utils import run_bass_kernel_spmd

bf16 = ml_dtypes.bfloat16

D_X, D_E, D_U = 128, 64, 64
D_HID, D_OUT = 256, 128
NB = 128   # nodes per block
WIN = 4    # 32-node one-hot windows per block
W = 32     # window width
KB = 8     # one-hot tiles generated per DVE op

FULL_CFG = dict(n_cores=8, n_nodes=100000, npc=12500, blocks=98, group=4)

_cache = {}


def _build_nc(Tb, blocks, npad, group, n_cores=8, reps=1):
    """Build the SPMD Bass program. Tb = per-slot edge tile counts.

    reps > 1 wraps the whole computation in a hardware For_i loop — used
    only for timing (per-iteration time = delta(wall)/delta(reps), which
    cancels the host dispatch overhead)."""
    Tb = list(Tb)
    nwin = blocks * WIN
    assert len(Tb) == nwin
    offs = [0]
    for t in Tb:
        offs.append(offs[-1] + t)
    TT = offs[-1]
    max_blk_tiles = max(
        sum(Tb[b * WIN : (b + 1) * WIN]) for b in range(blocks)
    )
    nc = bacc.Bacc(
        "TRN2", target_bir_lowering=False, debug=False, num_devices=n_cores
    )
    f32, rf32, b16 = mybir.dt.float32, mybir.dt.float32r, mybir.dt.bfloat16

    # partition-major: partition = edge slot within tile, free = (tile, feat)
    ea_in = nc.declare_dram_parameter("ea", [128, TT * 64], b16, isOutput=False)
    idx_in = nc.declare_dram_parameter("idx", [128, TT], b16, isOutput=False)
    iota_in = nc.declare_dram_parameter("iota", [128, KB, W], b16, isOutput=False)
    xT_in = nc.declare_dram_parameter("xT", [128, npad], b16, isOutput=False)
    vg_in = nc.declare_dram_parameter("vg", [1, npad], b16, isOutput=False)
    ones_in = nc.declare_dram_parameter("ones", [1, 64], b16, isOutput=False)
    iota64_in = nc.declare_dram_parameter("iota64", [64, 1], f32, isOutput=False)
    # weight layouts are partition-major: [K-part, mh, M]
    w1x_in = nc.declare_dram_parameter("w1x", [128, 2, 128], b16, isOutput=False)
    w1a_in = nc.declare_dram_parameter("w1a", [64, 2, 128], b16, isOutput=False)
    w1u_in = nc.declare_dram_parameter("w1u", [64, 2, 128], b16, isOutput=False)
    w2_in = nc.declare_dram_parameter("w2", [128, 2, 128], rf32, isOutput=False)
    b1_in = nc.declare_dram_parameter("b1", [128, 2], f32, isOutput=False)
    b2_in = nc.declare_dram_parameter("b2", [128, 1], f32, isOutput=False)
    outT = nc.declare_dram_parameter("outT", [128, npad], b16, isOutput=True)

    n_groups = (blocks + group - 1) // group
    GNB = group * NB

    with tile.TileContext(nc) as tc:
        with (
            tc.tile_pool(name="const", bufs=1) as cpool,
            tc.tile_pool(name="ea", bufs=6) as eapool,
            tc.tile_pool(name="p", bufs=6) as ppool,
            tc.tile_pool(name="hag", bufs=3) as hagpool,
            tc.tile_pool(name="ug", bufs=2) as ugpool,
            tc.tile_pool(name="h1", bufs=4) as h1pool,
            tc.tile_pool(name="outs", bufs=2) as opool,
            tc.tile_pool(name="ps_agg", bufs=2, space="PSUM") as agg_ps_pool,
            tc.tile_pool(name="ps_o1", bufs=2, space="PSUM") as o1_ps_pool,
            tc.tile_pool(name="ps_o2", bufs=2, space="PSUM") as o2_ps_pool,
            tc.tile_pool(name="ps_ugb", bufs=2, space="PSUM") as ugb_ps_pool,
        ):
          def _emit_body():
              # ---- constants / resident tensors (cheap Pool-ring DMAs) ----
              iota_t = cpool.tile([128, KB, W], b16, tag="iota")
              nc.gpsimd.dma_start(iota_t[:], iota_in[:])
              idx_t = cpool.tile([128, TT], b16, tag="idx")
              nc.gpsimd.dma_start(idx_t[:], idx_in[:])
              w1x_t = cpool.tile([128, 2, 128], b16, tag="w1x")
              nc.gpsimd.dma_start(w1x_t[:], w1x_in[:])
              w1a_t = cpool.tile([64, 2, 128], b16, tag="w1a")
              nc.gpsimd.dma_start(w1a_t[:], w1a_in[:])
              w1u_t = cpool.tile([64, 2, 128], b16, tag="w1u")
              nc.gpsimd.dma_start(w1u_t[:], w1u_in[:])
              w2_t = cpool.tile([128, 2, 128], rf32, tag="w2")
              nc.gpsimd.dma_start(w2_t[:], w2_in[:])
              b1_t = cpool.tile([128, 2], f32, tag="b1")
              nc.gpsimd.dma_start(b1_t[:], b1_in[:])
              b2_t = cpool.tile([128, 1], f32, tag="b2")
              nc.gpsimd.dma_start(b2_t[:], b2_in[:])
              vg_t = cpool.tile([1, npad], b16, tag="vg")
              nc.gpsimd.dma_start(vg_t[:], vg_in[:])
              ones_t = cpool.tile([1, 64], b16, tag="ones")
              nc.gpsimd.dma_start(ones_t[:], ones_in[:])
              iota64_t = cpool.tile([64, 1], f32, tag="iota64")
              nc.gpsimd.dma_start(iota64_t[:], iota64_in[:])

              xT_t = cpool.tile([128, npad], b16, tag="xT")
              # load x in chunks so early groups can start sooner
              xchunk = 8 * NB
              for s in range(0, npad, xchunk):
                  e = min(s + xchunk, npad)
                  nc.gpsimd.dma_start(xT_t[:, s:e], xT_in[:, s:e])

              hag_tiles = {}
              ug_tiles = {}
              p_cur = None

              def emit_mlp(g):
                  s = g * GNB
                  gw = min(GNB, npad - s)
                  hag = hag_tiles.pop(g)
                  ug_ps = ugb_ps_pool.tile([64, GNB], f32, tag="ugb")
                  nc.tensor.matmul(
                      ug_ps[:, :gw], ones_t[:], vg_t[:, s : s + gw],
                      start=True, stop=True,
                  )
                  p_u = ugpool.tile([64, GNB], b16, tag="pu")
                  nc.vector.tensor_scalar(
                      out=p_u[:, :gw], in0=ug_ps[:, :gw],
                      scalar1=iota64_t[:], scalar2=None,
                      op0=mybir.AluOpType.is_equal,
                  )
                  h1s = []
                  for mh in range(2):
                      o1 = o1_ps_pool.tile([128, GNB], f32, tag="o1")
                      nc.tensor.matmul(
                          o1[:, :gw], w1x_t[:, mh, :], xT_t[:, s : s + gw],
                          start=True, stop=False,
                      )
                      nc.tensor.matmul(
                          o1[:, :gw], w1a_t[:, mh, :], hag[:, :gw],
                          start=False, stop=False,
                      )
                      nc.tensor.matmul(
                          o1[:, :gw], w1u_t[:, mh, :], p_u[:, :gw],
                          start=False, stop=True,
                      )
                      h1 = h1pool.tile([128, GNB], rf32, tag="h1")
                      nc.scalar.activation(
                          out=h1[:, :gw], in_=o1[:, :gw],
                          func=mybir.ActivationFunctionType.Relu,
                          bias=b1_t[:, mh : mh + 1],
                      )
                      h1s.append(h1)
                  o2 = o2_ps_pool.tile([128, GNB], f32, tag="o2")
                  for kh in range(2):
                      nc.tensor.matmul(
                          o2[:, :gw], w2_t[:, kh, :], h1s[kh][:, :gw],
                          start=(kh == 0), stop=(kh == 1),
                      )
                  out_t = opool.tile([128, GNB], b16, tag="outs")
                  nc.scalar.activation(
                      out=out_t[:, :gw], in_=o2[:, :gw],
                      func=mybir.ActivationFunctionType.Identity,
                      bias=b2_t[:],
                  )
                  nc.scalar.dma_start(outT[:, s : s + gw], out_t[:, :gw])

              # ---- edge scatter-add per block, MLP interleaved per group ----
              ea_tiles = {}
              max_grp_tiles = max(
                  offs[min((g + 1) * group, blocks) * WIN] - offs[g * group * WIN]
                  for g in range(n_groups)
              )
              for b in range(blocks):
                  g, bi = divmod(b, group)
                  nblk_g = min(group, blocks - g * group)
                  if bi == 0:
                      hag_tiles[g] = hagpool.tile(
                          [64, GNB], b16, tag="hag", name=f"hag{g}"
                      )
                  o_g = offs[b * WIN]
                  Tblk = offs[(b + 1) * WIN] - o_g
                  ea_t = eapool.tile(
                      [128, max_blk_tiles * 64], b16, tag="ea", name=f"ea{b}"
                  )
                  ea_ring = nc.sync if b % 2 == 0 else nc.gpsimd
                  ea_ring.dma_start(
                      ea_t[:, : Tblk * 64],
                      ea_in[:, o_g * 64 : (o_g + Tblk) * 64],
                  )
                  agg_ps = agg_ps_pool.tile([64, NB], f32, tag="agg")
                  for w in range(WIN):
                      r = b * WIN + w
                      for t in range(Tb[r]):
                          o = offs[r] + t
                          if o % KB == 0:
                              kk = min(KB, TT - o)
                              p_cur = ppool.tile([128, KB, W], b16, tag="p")
                              idx_b = (
                                  idx_t[:, o : o + kk]
                                  .unsqueeze(2)
                                  .broadcast_to([128, kk, W])
                              )
                              nc.vector.scalar_tensor_tensor(
                                  out=p_cur[:, :kk, :],
                                  in0=iota_t[:, :kk, :],
                                  scalar=1.0,
                                  in1=idx_b,
                                  op0=mybir.AluOpType.mult,
                                  op1=mybir.AluOpType.is_equal,
                              )
                          nc.tensor.matmul(
                              agg_ps[:, W * w : W * (w + 1)],
                              ea_t[:, (o - o_g) * 64 : (o - o_g + 1) * 64],
                              p_cur[:, o % KB, :],
                              start=(t == 0),
                              stop=(t == Tb[r] - 1),
                          )
                  # stage [64, NB] agg into the group's MLP input tile (bf16)
                  nc.scalar.activation(
                      out=hag_tiles[g][:, bi * NB : (bi + 1) * NB],
                      in_=agg_ps[:],
                      func=mybir.ActivationFunctionType.Copy,
                  )
                  if bi == nblk_g - 1:
                      emit_mlp(g)

          if reps == 1:
              _emit_body()
          else:
              with tc.For_i(0, reps, 1):
                  _emit_body()

    nc.compile()
    return nc


def _pack_inputs(x, edge_index, edge_attr, u, v_indices, W1, b1, W2, b2, cfg):
    """Host-side sharding: bucket + sort edges by destination node window.

    Returns (in_maps, Tb, ids) where ids[c][j] is the global node id behind
    column j of core c's transposed layout (-1 for padding columns)."""
    n_cores, npc, blocks = cfg["n_cores"], cfg["npc"], cfg["blocks"]
    n_nodes = cfg["n_nodes"]
    npad = blocks * NB
    nwin = blocks * WIN
    row = np.asarray(edge_index[0], dtype=np.int64)
    ea = np.ascontiguousarray(np.asarray(edge_attr, dtype=np.float32))
    x = np.asarray(x, dtype=np.float32)
    u = np.asarray(u, dtype=np.float32)
    v_indices = np.asarray(v_indices, dtype=np.int64)
    W1 = np.asarray(W1, dtype=np.float32)
    W2 = np.asarray(W2, dtype=np.float32)
    b1 = np.asarray(b1, dtype=np.float32)
    b2 = np.asarray(b2, dtype=np.float32)
    d_e = ea.shape[1]

    order = np.argsort(row, kind="stable")
    row_s = row[order]
    ea_s = ea[order].astype(bf16)

    # window boundaries: core c window i covers nodes [npc*c + 32*i, +32),
    # clipped to the core's node range.
    bases = (npc * np.arange(n_cores)[:, None] + W * np.arange(nwin)[None, :]).ravel()
    core_hi = (npc * (1 + np.arange(n_cores))[:, None]).repeat(nwin, 1).ravel()
    starts = np.searchsorted(row_s, np.minimum(bases, core_hi), side="left")
    ends = np.searchsorted(row_s, np.minimum(bases + W, core_hi), side="left")
    cnts = (ends - starts).reshape(n_cores, nwin)

    # rank-match: each core sorts its windows by count desc; slot r on every
    # core holds that core's r-th largest window, so the shared Tb is tight.
    ordw = np.argsort(-cnts, axis=1, kind="stable")          # [n_cores, nwin]
    cnt_sorted = np.take_along_axis(cnts, ordw, axis=1)
    mx = cnt_sorted.max(axis=0)                               # [nwin]
    Tb = np.maximum(1, -(-mx // 128)).astype(int)
    offs = np.concatenate([[0], np.cumsum(Tb)])
    TT = int(offs[-1])

    uT = u.T  # [d_u, n_graphs]
    starts2 = starts.reshape(n_cores, nwin)
    ends2 = ends.reshape(n_cores, nwin)

    in_maps = []
    ids_list = []
    iota = np.broadcast_to(
        np.arange(W, dtype=np.float32), (128, KB, W)
    ).astype(bf16)
    # weights, partition-major [K, mh, M]
    w1x = np.ascontiguousarray(W1[:D_X].reshape(D_X, 2, 128)).astype(bf16)
    w1a = np.ascontiguousarray(W1[D_X : D_X + d_e].reshape(d_e, 2, 128)).astype(bf16)
    w1u = np.ascontiguousarray((u @ W1[D_X + d_e :]).reshape(u.shape[0], 2, 128)).astype(bf16)
    w2 = np.ascontiguousarray(W2.reshape(2, 128, D_OUT).transpose(1, 0, 2))
    b1p = np.ascontiguousarray(b1.reshape(2, 128).T)
    b2p = np.ascontiguousarray(b2.reshape(128, 1))

    for c in range(n_cores):
        cnt = cnts[c]
        cs, ce = starts2[c, 0], ends2[c, -1]
        slotof = np.empty(nwin, dtype=np.int64)
        slotof[ordw[c]] = np.arange(nwin)
        w_e = np.repeat(np.arange(nwin), cnt)            # window id per edge
        rank = np.arange(ce - cs) - np.repeat(starts2[c] - cs, cnt)
        slot_idx = offs[slotof[w_e]] * 128 + rank
        coreslots = np.zeros((TT * 128, d_e), dtype=bf16)
        coreslots[slot_idx] = ea_s[cs:ce]
        ea_pack = (
            coreslots.reshape(TT, 128, d_e).transpose(1, 0, 2).reshape(128, TT * d_e)
        )
        ivals = np.zeros(TT * 128, dtype=np.float32)
        ivals[slot_idx] = (row_s[cs:ce] - (npc * c + W * w_e)).astype(np.float32)
        idx_pack = np.ascontiguousarray(ivals.reshape(TT, 128).T).astype(bf16)

        base_nodes = npc * c + W * ordw[c]                # [nwin]
        ids = (base_nodes[:, None] + np.arange(W)).ravel()  # [npad]
        valid = ids < min(npc * (c + 1), n_nodes)
        ids_eff = np.where(valid, ids, 0)
        xT = np.where(valid[None, :], x[ids_eff].T, 0.0).astype(bf16)
        vg = np.where(valid, v_indices[ids_eff], 0).astype(np.float32)[None, :]
        ids_list.append(np.where(valid, ids, -1))
        in_maps.append({
            "ea": ea_pack,
            "idx": idx_pack,
            "iota": iota,
            "xT": np.ascontiguousarray(xT),
            "vg": vg.astype(bf16),
            "ones": np.ones((1, 64), dtype=bf16),
            "iota64": np.arange(64, dtype=np.float32).reshape(64, 1),
            "w1x": w1x,
            "w1a": w1a,
            "w1u": w1u,
            "w2": w2,
            "b1": b1p,
            "b2": b2p,
        })
    return in_maps, tuple(int(t) for t in Tb), ids_list


def unpack_out(outT_list, ids_list, n_nodes=100000):
    out = np.empty((n_nodes, D_OUT), dtype=np.float32)
    for c, ids in enumerate(ids_list):
        valid = ids >= 0
        out[ids[valid]] = outT_list[c].T[valid].astype(np.float32)
    return out


def _run(inputs, cfg, trace=False, reps=1):
    in_maps, T, ids_list = _pack_inputs(
        inputs["x"], inputs["edge_index"], inputs["edge_attr"], inputs["u"],
        inputs["v_indices"], inputs["W1"], inputs["b1"], inputs["W2"],
        inputs["b2"], cfg,
    )
    key = (T, cfg["blocks"], cfg["group"], reps)
    if key not in _cache:
        _cache[key] = _build_nc(
            T, cfg["blocks"], cfg["blocks"] * NB, cfg["group"], reps=reps
        )
    nc = _cache[key]
    res = run_bass_kernel_spmd(nc, in_maps, list(range(cfg["n_cores"])), trace=trace)
    out = unpack_out(
        [res.results[c]["outT"] for c in range(cfg["n_cores"])],
        ids_list, cfg["n_nodes"],
    )
    _run.last_results = res
    return out


def kernel(x, edge_index, edge_attr, u, v_indices, W1, b1, W2, b2):
    inputs = dict(x=x, edge_index=edge_index, edge_attr=edge_attr, u=u,
                  v_indices=v_indices, W1=W1, b1=b1, W2=W2, b2=b2)
    return _run(inputs, FULL_CFG)


# revision 15
# speedup vs baseline: 1.3202x; 1.1016x over previous
"""Trainium2 Bass kernel for nn_NodeModel (GNN message passing).

reference:
    agg = segment_sum(edge_attr, edge_index[0], num_segments=100000)   # [N, 64]
    h = concat([x, agg, u[v_indices]], axis=1)                         # [N, 256]
    out = relu(h @ W1 + b1) @ W2 + b2                                  # [N, 128]

Strategy (8 NeuronCores, SPMD, no collectives):
  - Shard nodes across cores (12500/core); shard edges by destination-node
    partition (host buckets+sorts edges by the core/window owning their row).
  - Nodes are processed in blocks of 128 (4 windows of 32). Edges are sorted
    by row, grouped per 32-node window, padded to Tb[slot] tiles of 128 edges.
    Since all 8 cores share one program, Tb is the max over cores; to cut the
    padding, each core's windows are rank-matched (sorted by edge count) so
    Tb[r] = ceil(max_c sorted_cnt[c][r]/128). The window->slot permutation is
    un-done on the host (x/u/out columns follow the same per-core permutation).
  - segment_sum on device: per 128-edge tile, one-hot P[e, m] = (idx[e] == m)
    built in batches of KB=8 tiles with ONE DVE op (broadcast APs), then
    TensorE matmul aggT += ea_tile.T @ P accumulated in PSUM.
  - Everything flows in bf16 (edge_attr, x, u-gather, W1) except the f32
    PSUM accumulators, the W2 matmul (fp32r) and the f32 output; rel err vs
    the f32 reference is ~5e-3, well under the 2e-2 gate.
  - MLP runs feature-major (transposed), N=512 node groups, interleaved with
    the edge phase (group g's MLP is emitted right after its 4 blocks) so
    there is no serial MLP tail. Output is un-transposed on host.
"""

import sys

sys.path.insert(0, "/opt/trn_rl_repo")

import numpy as np
import ml_dtypes

import concourse.bass as bass
import concourse.mybir as mybir
from concourse import bacc, tile
from concourse.bass_utils import run_bass_kernel_spmd

bf16 = ml_dtypes.bfloat16

D_X, D_E, D_U = 128, 64, 64
D_HID, D_OUT = 256, 128
NB = 128   # nodes per block
WIN = 4    # 32-node one-hot windows per block
W = 32     # window width
KB = 8     # one-hot tiles generated per DVE op

FULL_CFG = dict(n_cores=8, n_nodes=100000, npc=12500, blocks=98, group=4)

_cache = {}


def _build_nc(Tb, blocks, npad, group, n_cores=8, reps=1):
    """Build the SPMD Bass program. Tb = per-slot edge tile counts.

    reps > 1 wraps the whole computation in a hardware For_i loop — used
    only for timing (per-iteration time = delta(wall)/delta(reps), which
    cancels the host dispatch overhead)."""
    Tb = list(Tb)
    nwin = blocks * WIN
    assert len(Tb) == nwin
    offs = [0]
    for t in Tb:
        offs.append(offs[-1] + t)
    TT = offs[-1]
    max_blk_tiles = max(
        sum(Tb[b * WIN : (b + 1) * WIN]) for b in range(blocks)
    )
    nc = bacc.Bacc(
        "TRN2", target_bir_lowering=False, debug=False, num_devices=n_cores
    )
    f32, rf32, b16 = mybir.dt.float32, mybir.dt.float32r, mybir.dt.bfloat16

    # partition-major: partition = edge slot within tile, free = (tile, feat)
    ea_in = nc.declare_dram_parameter("ea", [128, TT * 64], b16, isOutput=False)
    idx_in = nc.declare_dram_parameter("idx", [128, TT], b16, isOutput=False)
    iota_in = nc.declare_dram_parameter("iota", [128, KB, W], b16, isOutput=False)
    xT_in = nc.declare_dram_parameter("xT", [128, npad], b16, isOutput=False)
    ugT_in = nc.declare_dram_parameter("ugT", [64, npad], b16, isOutput=False)
    # weight layouts are partition-major: [K-part, mh, M]
    w1x_in = nc.declare_dram_parameter("w1x", [128, 2, 128], b16, isOutput=False)
    w1a_in = nc.declare_dram_parameter("w1a", [64, 2, 128], b16, isOutput=False)
    w1u_in = nc.declare_dram_parameter("w1u", [64, 2, 128], b16, isOutput=False)
    w2_in = nc.declare_dram_parameter("w2", [128, 2, 128], rf32, isOutput=False)
    b1_in = nc.declare_dram_parameter("b1", [128, 2], f32, isOutput=False)
    b2_in = nc.declare_dram_parameter("b2", [128, 1], f32, isOutput=False)
    outT = nc.declare_dram_parameter("outT", [128, npad], b16, isOutput=True)

    n_groups = (blocks + group - 1) // group
    GNB = group * NB

    with tile.TileContext(nc) as tc:
        with (
            tc.tile_pool(name="const", bufs=1) as cpool,
            tc.tile_pool(name="ea", bufs=6) as eapool,
            tc.tile_pool(name="p", bufs=6) as ppool,
            tc.tile_pool(name="hag", bufs=3) as hagpool,
            tc.tile_pool(name="ug", bufs=2) as ugpool,
            tc.tile_pool(name="h1", bufs=4) as h1pool,
            tc.tile_pool(name="outs", bufs=2) as opool,
            tc.tile_pool(name="ps_agg", bufs=4, space="PSUM") as agg_ps_pool,
            tc.tile_pool(name="ps_o1", bufs=2, space="PSUM") as o1_ps_pool,
            tc.tile_pool(name="ps_o2", bufs=2, space="PSUM") as o2_ps_pool,
        ):
          def _emit_body():
              # ---- constants / resident tensors (cheap Pool-ring DMAs) ----
              iota_t = cpool.tile([128, KB, W], b16, tag="iota")
              nc.gpsimd.dma_start(iota_t[:], iota_in[:])
              idx_t = cpool.tile([128, TT], b16, tag="idx")
              nc.gpsimd.dma_start(idx_t[:], idx_in[:])
              w1x_t = cpool.tile([128, 2, 128], b16, tag="w1x")
              nc.gpsimd.dma_start(w1x_t[:], w1x_in[:])
              w1a_t = cpool.tile([64, 2, 128], b16, tag="w1a")
              nc.gpsimd.dma_start(w1a_t[:], w1a_in[:])
              w1u_t = cpool.tile([64, 2, 128], b16, tag="w1u")
              nc.gpsimd.dma_start(w1u_t[:], w1u_in[:])
              w2_t = cpool.tile([128, 2, 128], rf32, tag="w2")
              nc.gpsimd.dma_start(w2_t[:], w2_in[:])
              b1_t = cpool.tile([128, 2], f32, tag="b1")
              nc.gpsimd.dma_start(b1_t[:], b1_in[:])
              b2_t = cpool.tile([128, 1], f32, tag="b2")
              nc.gpsimd.dma_start(b2_t[:], b2_in[:])

              xT_t = cpool.tile([128, npad], b16, tag="xT")
              # load x in chunks so early groups can start sooner
              xchunk = 8 * NB
              for s in range(0, npad, xchunk):
                  e = min(s + xchunk, npad)
                  nc.gpsimd.dma_start(xT_t[:, s:e], xT_in[:, s:e])

              hag_tiles = {}
              ug_tiles = {}
              p_cur = None

              def emit_mlp(g):
                  s = g * GNB
                  gw = min(GNB, npad - s)
                  hag = hag_tiles.pop(g)
                  ug_t = ug_tiles.pop(g)
                  h1s = []
                  for mh in range(2):
                      o1 = o1_ps_pool.tile([128, GNB], f32, tag="o1")
                      nc.tensor.matmul(
                          o1[:, :gw], w1x_t[:, mh, :], xT_t[:, s : s + gw],
                          start=True, stop=False,
                      )
                      nc.tensor.matmul(
                          o1[:, :gw], w1a_t[:, mh, :], hag[:, :gw],
                          start=False, stop=False,
                      )
                      nc.tensor.matmul(
                          o1[:, :gw], w1u_t[:, mh, :], ug_t[:, :gw],
                          start=False, stop=True,
                      )
                      h1 = h1pool.tile([128, GNB], rf32, tag="h1")
                      nc.scalar.activation(
                          out=h1[:, :gw], in_=o1[:, :gw],
                          func=mybir.ActivationFunctionType.Relu,
                          bias=b1_t[:, mh : mh + 1],
                      )
                      h1s.append(h1)
                  o2 = o2_ps_pool.tile([128, GNB], f32, tag="o2")
                  for kh in range(2):
                      nc.tensor.matmul(
                          o2[:, :gw], w2_t[:, kh, :], h1s[kh][:, :gw],
                          start=(kh == 0), stop=(kh == 1),
                      )
                  out_t = opool.tile([128, GNB], b16, tag="outs")
                  nc.scalar.activation(
                      out=out_t[:, :gw], in_=o2[:, :gw],
                      func=mybir.ActivationFunctionType.Identity,
                      bias=b2_t[:],
                  )
                  nc.scalar.dma_start(outT[:, s : s + gw], out_t[:, :gw])

              # ---- edge scatter-add per block, MLP interleaved per group ----
              ea_tiles = {}
              max_grp_tiles = max(
                  offs[min((g + 1) * group, blocks) * WIN] - offs[g * group * WIN]
                  for g in range(n_groups)
              )
              for b in range(blocks):
                  g, bi = divmod(b, group)
                  nblk_g = min(group, blocks - g * group)
                  if bi == 0:
                      hag_tiles[g] = hagpool.tile(
                          [64, GNB], b16, tag="hag", name=f"hag{g}"
                      )
                      s = g * GNB
                      gw = min(GNB, npad - s)
                      ug_t = ugpool.tile([64, GNB], b16, tag="ug", name=f"ug{g}")
                      nc.gpsimd.dma_start(ug_t[:, :gw], ugT_in[:, s : s + gw])
                      ug_tiles[g] = ug_t
                  o_g = offs[b * WIN]
                  Tblk = offs[(b + 1) * WIN] - o_g
                  ea_t = eapool.tile(
                      [128, max_blk_tiles * 64], b16, tag="ea", name=f"ea{b}"
                  )
                  ea_ring = nc.sync if b % 2 == 0 else nc.gpsimd
                  ea_ring.dma_start(
                      ea_t[:, : Tblk * 64],
                      ea_in[:, o_g * 64 : (o_g + Tblk) * 64],
                  )
                  agg_ps = agg_ps_pool.tile([64, NB], f32, tag="agg")
                  for w in range(WIN):
                      r = b * WIN + w
                      for t in range(Tb[r]):
                          o = offs[r] + t
                          if o % KB == 0:
                              kk = min(KB, TT - o)
                              p_cur = ppool.tile([128, KB, W], b16, tag="p")
                              idx_b = (
                                  idx_t[:, o : o + kk]
                                  .unsqueeze(2)
                                  .broadcast_to([128, kk, W])
                              )
                              nc.vector.scalar_tensor_tensor(
                                  out=p_cur[:, :kk, :],
                                  in0=iota_t[:, :kk, :],
                                  scalar=1.0,
                                  in1=idx_b,
                                  op0=mybir.AluOpType.mult,
                                  op1=mybir.AluOpType.is_equal,
                              )
                          nc.tensor.matmul(
                              agg_ps[:, W * w : W * (w + 1)],
                              ea_t[:, (o - o_g) * 64 : (o - o_g + 1) * 64],
                              p_cur[:, o % KB, :],
                              start=(t == 0),
                              stop=(t == Tb[r] - 1),
                          )
                  # stage [64, NB] agg into the group's MLP input tile (bf16)
                  nc.scalar.activation(
                      out=hag_tiles[g][:, bi * NB : (bi + 1) * NB],
                      in_=agg_ps[:],
                      func=mybir.ActivationFunctionType.Copy,
                  )
                  if bi == nblk_g - 1:
                      emit_mlp(g)

          if reps == 1:
              _emit_body()
          else:
              with tc.For_i(0, reps, 1):
                  _emit_body()

    nc.compile()
    return nc


def _pack_inputs(x, edge_index, edge_attr, u, v_indices, W1, b1, W2, b2, cfg):
    """Host-side sharding: bucket + sort edges by destination node window.

    Returns (in_maps, Tb, ids) where ids[c][j] is the global node id behind
    column j of core c's transposed layout (-1 for padding columns)."""
    n_cores, npc, blocks = cfg["n_cores"], cfg["npc"], cfg["blocks"]
    n_nodes = cfg["n_nodes"]
    npad = blocks * NB
    nwin = blocks * WIN
    row = np.asarray(edge_index[0], dtype=np.int64)
    ea = np.ascontiguousarray(np.asarray(edge_attr, dtype=np.float32))
    x = np.asarray(x, dtype=np.float32)
    u = np.asarray(u, dtype=np.float32)
    v_indices = np.asarray(v_indices, dtype=np.int64)
    W1 = np.asarray(W1, dtype=np.float32)
    W2 = np.asarray(W2, dtype=np.float32)
    b1 = np.asarray(b1, dtype=np.float32)
    b2 = np.asarray(b2, dtype=np.float32)
    d_e = ea.shape[1]

    order = np.argsort(row, kind="stable")
    row_s = row[order]
    ea_s = ea[order].astype(bf16)

    # window boundaries: core c window i covers nodes [npc*c + 32*i, +32),
    # clipped to the core's node range.
    bases = (npc * np.arange(n_cores)[:, None] + W * np.arange(nwin)[None, :]).ravel()
    core_hi = (npc * (1 + np.arange(n_cores))[:, None]).repeat(nwin, 1).ravel()
    starts = np.searchsorted(row_s, np.minimum(bases, core_hi), side="left")
    ends = np.searchsorted(row_s, np.minimum(bases + W, core_hi), side="left")
    cnts = (ends - starts).reshape(n_cores, nwin)

    # rank-match: each core sorts its windows by count desc; slot r on every
    # core holds that core's r-th largest window, so the shared Tb is tight.
    ordw = np.argsort(-cnts, axis=1, kind="stable")          # [n_cores, nwin]
    cnt_sorted = np.take_along_axis(cnts, ordw, axis=1)
    mx = cnt_sorted.max(axis=0)                               # [nwin]
    Tb = np.maximum(1, -(-mx // 128)).astype(int)
    offs = np.concatenate([[0], np.cumsum(Tb)])
    TT = int(offs[-1])

    uT = u.T  # [d_u, n_graphs]
    starts2 = starts.reshape(n_cores, nwin)
    ends2 = ends.reshape(n_cores, nwin)

    in_maps = []
    ids_list = []
    iota = np.broadcast_to(
        np.arange(W, dtype=np.float32), (128, KB, W)
    ).astype(bf16)
    # weights, partition-major [K, mh, M]
    w1x = np.ascontiguousarray(W1[:D_X].reshape(D_X, 2, 128)).astype(bf16)
    w1a = np.ascontiguousarray(W1[D_X : D_X + d_e].reshape(d_e, 2, 128)).astype(bf16)
    w1u = np.ascontiguousarray(W1[D_X + d_e :].reshape(D_U, 2, 128)).astype(bf16)
    w2 = np.ascontiguousarray(W2.reshape(2, 128, D_OUT).transpose(1, 0, 2))
    b1p = np.ascontiguousarray(b1.reshape(2, 128).T)
    b2p = np.ascontiguousarray(b2.reshape(128, 1))

    for c in range(n_cores):
        cnt = cnts[c]
        cs, ce = starts2[c, 0], ends2[c, -1]
        slotof = np.empty(nwin, dtype=np.int64)
        slotof[ordw[c]] = np.arange(nwin)
        w_e = np.repeat(np.arange(nwin), cnt)            # window id per edge
        rank = np.arange(ce - cs) - np.repeat(starts2[c] - cs, cnt)
        slot_idx = offs[slotof[w_e]] * 128 + rank
        coreslots = np.zeros((TT * 128, d_e), dtype=bf16)
        coreslots[slot_idx] = ea_s[cs:ce]
        ea_pack = (
            coreslots.reshape(TT, 128, d_e).transpose(1, 0, 2).reshape(128, TT * d_e)
        )
        ivals = np.zeros(TT * 128, dtype=np.float32)
        ivals[slot_idx] = (row_s[cs:ce] - (npc * c + W * w_e)).astype(np.float32)
        idx_pack = np.ascontiguousarray(ivals.reshape(TT, 128).T).astype(bf16)

        base_nodes = npc * c + W * ordw[c]                # [nwin]
        ids = (base_nodes[:, None] + np.arange(W)).ravel()  # [npad]
        valid = ids < min(npc * (c + 1), n_nodes)
        ids_eff = np.where(valid, ids, 0)
        xT = np.where(valid[None, :], x[ids_eff].T, 0.0).astype(bf16)
        ugT = np.where(valid[None, :], uT[:, v_indices[ids_eff]], 0.0).astype(bf16)
        ids_list.append(np.where(valid, ids, -1))
        in_maps.append({
            "ea": ea_pack,
            "idx": idx_pack,
            "iota": iota,
            "xT": np.ascontiguousarray(xT),
            "ugT": np.ascontiguousarray(ugT),
            "w1x": w1x,
            "w1a": w1a,
            "w1u": w1u,
            "w2": w2,
            "b1": b1p,
            "b2": b2p,
        })
    return in_maps, tuple(int(t) for t in Tb), ids_list


def unpack_out(outT_list, ids_list, n_nodes=100000):
    out = np.empty((n_nodes, D_OUT), dtype=np.float32)
    for c, ids in enumerate(ids_list):
        valid = ids >= 0
        out[ids[valid]] = outT_list[c].T[valid].astype(np.float32)
    return out


def _run(inputs, cfg, trace=False, reps=1):
    in_maps, T, ids_list = _pack_inputs(
        inputs["x"], inputs["edge_index"], inputs["edge_attr"], inputs["u"],
        inputs["v_indices"], inputs["W1"], inputs["b1"], inputs["W2"],
        inputs["b2"], cfg,
    )
    key = (T, cfg["blocks"], cfg["group"], reps)
    if key not in _cache:
        _cache[key] = _build_nc(
            T, cfg["blocks"], cfg["blocks"] * NB, cfg["group"], reps=reps
        )
    nc = _cache[key]
    res = run_bass_kernel_spmd(nc, in_maps, list(range(cfg["n_cores"])), trace=trace)
    out = unpack_out(
        [res.results[c]["outT"] for c in range(cfg["n_cores"])],
        ids_list, cfg["n_nodes"],
    )
    _run.last_results = res
    return out


def kernel(x, edge_index, edge_attr, u, v_indices, W1, b1, W2, b2):
    inputs = dict(x=x, edge_index=edge_index, edge_attr=edge_attr, u=u,
                  v_indices=v_indices, W1=W1, b1=b1, W2=W2, b2=b2)
    return _run(inputs, FULL_CFG)
